# revision 1
# baseline (speedup 1.0000x reference)
"""Trainium2 Bass kernel for nn_EventPairCompositionModel.

Strategy (data-parallel over batch, 8 cores, B=512 -> 64 per core):
  - Host compacts the 60MB f32 table per core to the ~24K unique rows its
    shard touches (bf16, rows padded to 384 elems = 768B), remapping all
    indices to int16.  The device then uses the fast SWDGE dma_gather
    (InstDMAGatherAnt) to fetch context/event embeddings.
  - XBAR DMA transpose (SBUF->SBUF) turns gathered bn-major rows into
    K-major tiles for the tensor engine.
  - Shared arg-composition MLP (1536->512->256, zero-padded K) in bf16.
  - Cosine numerators/denominators via small per-b matmuls that land
    n-on-partitions; norms folded together through one exp(-0.5 ln x).
  - KNRM kernel pooling via ones-matmul partition reductions, distance
    kernel path, final linear + sigmoid, all on-chip.
  - If a shard ever touches >32767 unique rows (can't happen for random
    inputs), falls back to a slow indirect-DMA gather of the full table.
All 8 cores run the identical program on their own batch shard (SPMD, no
collectives); host concatenates the 8 (64,1) outputs.
"""

import numpy as np
import ml_dtypes

import concourse.bacc as bacc
import concourse.bass as bass
import concourse.tile as tile
import concourse.mybir as mybir
from concourse.bass import IndirectOffsetOnAxis
from concourse.bass_utils import run_bass_kernel_spmd
from concourse import library_config

F32 = mybir.dt.float32
BF16 = mybir.dt.bfloat16
I16 = mybir.dt.int16
I32 = mybir.dt.int32
AF = mybir.ActivationFunctionType

# Problem shapes (hardcoded per spec)
B, N, C, E = 512, 128, 4, 300
V = 50000
H1, H2 = 512, 256
NF, NK = 8, 11
NCORES = 8
BC = B // NCORES          # 64 batches per core
EP = 384                  # padded embedding stride inside an x-row (768B)
CE = C * EP               # 1536 padded x-row length
KT = CE // 128            # 12 K-tiles for MLP1
CT = 32768                # compact table rows (int16-indexable)
GROUPS = (BC * N) // 512  # 16 groups of 512 (b,n) pairs
SUBT = 4                  # 128-bn subtiles per group
EB = 128                  # event-path width (64 real b + 64 junk)

MUS = [1.0, 0.9, 0.7, 0.5, 0.3, 0.1, -0.1, -0.3, -0.5, -0.7, -0.9]
SIGMAS = [1e-3] + [0.1] * 10

_PROGRAM_CACHE = {}


def _build_program(fast: bool):
    if fast in _PROGRAM_CACHE:
        return _PROGRAM_CACHE[fast]

    nc = bacc.Bacc("TRN2", target_bir_lowering=False, debug=False, num_swdge_queues=4)

    # ---- DRAM I/O ----
    if fast:
        ctab = nc.dram_tensor("ctab", (CT, EP), BF16, kind="ExternalInput")
        cidx = nc.dram_tensor("cidx", (128, GROUPS * 128), I16, kind="ExternalInput")
        eidx = nc.dram_tensor("eidx", (128, 32), I16, kind="ExternalInput")
    else:
        ctab = nc.dram_tensor("table", (V + 1, E), F32, kind="ExternalInput")
        cidx = nc.dram_tensor("ctxidx", (128, BC * C), I32, kind="ExternalInput")
        eidx = nc.dram_tensor("evidx", (BC, C), I32, kind="ExternalInput")
    w1t = nc.dram_tensor("w1t", (CE, H1), BF16, kind="ExternalInput")
    w2t = nc.dram_tensor("w2t", (H1, H2), BF16, kind="ExternalInput")
    wvt = nc.dram_tensor("wvt", (CE, 9), BF16, kind="ExternalInput")
    b1d = nc.dram_tensor("b1d", (128, 4), F32, kind="ExternalInput")
    b2d = nc.dram_tensor("b2d", (128, 2), F32, kind="ExternalInput")
    bvd = nc.dram_tensor("bvd", (9, 1), F32, kind="ExternalInput")
    wct = nc.dram_tensor("wct", (128, 1), F32, kind="ExternalInput")
    wckp = nc.dram_tensor("wckp", (1, NK), F32, kind="ExternalInput")
    bcd = nc.dram_tensor("bcd", (1, 1), F32, kind="ExternalInput")
    ndsq = nc.dram_tensor("ndsq", (9, BC), F32, kind="ExternalInput")
    featT = nc.dram_tensor("featT", (NF, BC), F32, kind="ExternalInput")
    out_d = nc.dram_tensor("out", (BC, 1), F32, kind="ExternalOutput")

    with tile.TileContext(nc) as tc:
        with (
            tc.tile_pool(name="consts", bufs=1) as cpool,
            tc.tile_pool(name="xg", bufs=4) as xgpool,
            tc.tile_pool(name="xt", bufs=4) as xtpool,
            tc.tile_pool(name="s1", bufs=8) as s1pool,
            tc.tile_pool(name="s2", bufs=4) as s2pool,
            tc.tile_pool(name="csq", bufs=4) as csqpool,
            tc.tile_pool(name="small", bufs=2) as smpool,
            tc.tile_pool(name="pm1", bufs=2, space="PSUM") as pm1,
            tc.tile_pool(name="pm2", bufs=2, space="PSUM") as pm2,
            tc.tile_pool(name="ptn", bufs=1, space="PSUM") as ptn,
            tc.tile_pool(name="pmisc", bufs=2, space="PSUM") as pmisc,
        ):
            # ---- load constants ----
            if fast:
                nc.gpsimd.load_library(library_config.mlp)
                cidx_s = cpool.tile([128, GROUPS * 128], I16)
                nc.sync.dma_start(cidx_s[:], cidx.ap())
                eidx_s = cpool.tile([128, 32], I16)
                nc.sync.dma_start(eidx_s[:], eidx.ap())
            w1t_s = cpool.tile([128, KT * H1], BF16)
            nc.sync.dma_start(
                w1t_s[:].rearrange("p (t m) -> p t m", t=KT),
                w1t.ap().rearrange("(t p) m -> p t m", p=128),
            )
            w2t_s = cpool.tile([128, 4 * H2], BF16)
            nc.scalar.dma_start(
                w2t_s[:].rearrange("p (t m) -> p t m", t=4),
                w2t.ap().rearrange("(t p) m -> p t m", p=128),
            )
            wvt_s = cpool.tile([128, KT * 9], BF16)
            nc.scalar.dma_start(
                wvt_s[:].rearrange("p (t m) -> p t m", t=KT),
                wvt.ap().rearrange("(t p) m -> p t m", p=128),
            )
            b1_s = cpool.tile([128, 4], F32)
            nc.sync.dma_start(b1_s[:], b1d.ap())
            b2_s = cpool.tile([128, 2], F32)
            nc.sync.dma_start(b2_s[:], b2d.ap())
            bv_s = cpool.tile([9, 1], F32)
            nc.sync.dma_start(bv_s[:], bvd.ap())
            wct_s = cpool.tile([128, 1], F32)
            nc.sync.dma_start(wct_s[:], wct.ap())
            wckp_s = cpool.tile([1, NK], F32)
            nc.sync.dma_start(wckp_s[:], wckp.ap())
            bc_s = cpool.tile([1, 1], F32)
            nc.sync.dma_start(bc_s[:], bcd.ap())
            if not fast:
                cidx_s = cpool.tile([128, BC * C], I32)
                nc.sync.dma_start(cidx_s[:], cidx.ap())
                eidx_s = cpool.tile([BC, C], I32)
                nc.sync.dma_start(eidx_s[:], eidx.ap())
            ndsq_s = cpool.tile([9, BC], F32)
            nc.sync.dma_start(ndsq_s[:], ndsq.ap())
            feat_s = cpool.tile([128, BC], F32)
            nc.vector.memset(feat_s[:], 0.0)
            nc.sync.dma_start(feat_s[64 : 64 + NF, :], featT.ap())
            ones_s = cpool.tile([128, 1], BF16)
            nc.vector.memset(ones_s[:], 1.0)
            onesrow_s = cpool.tile([1, 128], F32)
            nc.vector.memset(onesrow_s[:], 1.0)
            onesf_s = cpool.tile([128, 1], F32)
            nc.vector.memset(onesf_s[:], 1.0)
            eps_s = cpool.tile([128, 1], F32)
            nc.vector.memset(eps_s[:], 1e-20)
            mub_s = cpool.tile([128, NK], F32)
            for k in range(NK):
                nc.vector.memset(mub_s[:, k : k + 1], -MUS[k])

            # ---- event path (EB=128 lanes, only 0..63 meaningful) ----
            xeT = cpool.tile([128, KT * EB], BF16)
            if fast:
                # transpose-mode gather lands K-major directly:
                # xeT[p, jj, c*128+b] = emb_{b,c}[jj*128+p]
                nc.gpsimd.dma_gather(
                    out_ap=xeT[:].rearrange("p (j i) -> p j i", j=3),
                    in_ap=ctab.ap(),
                    idxs_ap=eidx_s[:],
                    num_idxs=512,
                    num_idxs_reg=512,
                    elem_size=EP,
                    transpose=True,
                )
            else:
                xe = cpool.tile([EB, CE], BF16)
                nc.vector.memset(xe[:], 0.0)
                nc.gpsimd.indirect_dma_start(
                    out=xe[0:BC, :].rearrange("p (c e) -> p c e", c=C)[:, :, 0:E],
                    out_offset=None,
                    in_=ctab.ap(),
                    in_offset=IndirectOffsetOnAxis(ap=eidx_s[:], axis=0),
                )
                nc.sync.dma_start_transpose(
                    xeT[:].rearrange("p (j i) -> p j i", j=KT), xe[:]
                )

            def xeT_k(j):
                # K-tile j = 3*c + jj of the event activations
                if fast:
                    return xeT[:, 512 * (j % 3) + 128 * (j // 3) :][:, 0:128]
                return xeT[:, EB * j : EB * (j + 1)]

            s1e = cpool.tile([128, 4 * EB], BF16)
            for m in range(4):
                pe = pmisc.tile([128, EB], F32, tag="pmisc", name="pe")
                for j in range(KT):
                    nc.tensor.matmul(
                        pe[:],
                        w1t_s[:, H1 * j + 128 * m : H1 * j + 128 * m + 128],
                        xeT_k(j),
                        start=(j == 0),
                        stop=(j == KT - 1),
                    )
                nc.scalar.activation(
                    s1e[:, EB * m : EB * (m + 1)], pe[:], AF.Relu,
                    bias=b1_s[:, m : m + 1],
                )

            eh2 = [
                cpool.tile([128, EB], BF16, tag=f"eh2_{k}", name=f"eh2_{k}")
                for k in range(2)
            ]
            for m in range(2):
                pe2 = pmisc.tile([128, EB], F32, tag="pmisc", name="pe2")
                for j in range(4):
                    nc.tensor.matmul(
                        pe2[:],
                        w2t_s[:, H2 * j + 128 * m : H2 * j + 128 * m + 128],
                        s1e[:, EB * j : EB * (j + 1)],
                        start=(j == 0),
                        stop=(j == 3),
                    )
                nc.scalar.activation(
                    eh2[m][:], pe2[:], AF.Relu, bias=b2_s[:, m : m + 1]
                )

            # variances -> dist_emb rows 32..40 of feat_s
            pv = pmisc.tile([9, EB], F32, tag="pmisc", name="pv")
            for j in range(KT):
                nc.tensor.matmul(
                    pv[:],
                    wvt_s[:, 9 * j : 9 * (j + 1)],
                    xeT_k(j),
                    start=(j == 0),
                    stop=(j == KT - 1),
                )
            ez_s = smpool.tile([9, EB], F32)
            nc.scalar.activation(ez_s[:], pv[:], AF.Exp, bias=bv_s[:])
            ez1_s = smpool.tile([9, EB], F32)
            nc.vector.tensor_scalar_add(ez1_s[:], ez_s[:], 1.0)
            var_s = smpool.tile([9, EB], F32)
            nc.scalar.activation(var_s[:], ez1_s[:], AF.Ln)
            rv_s = smpool.tile([9, EB], F32)
            nc.vector.reciprocal(rv_s[:], var_s[:])
            q_s = smpool.tile([9, BC], F32)
            nc.vector.tensor_mul(q_s[:], ndsq_s[:], rv_s[:, 0:BC])
            nc.scalar.activation(feat_s[32:41, :], q_s[:], AF.Exp)

            # |e|^2 per b, broadcast to all 128 partitions via outer product
            esq = [
                smpool.tile([128, EB], BF16, tag=f"esq_{k}", name=f"esq_{k}")
                for k in range(2)
            ]
            for k in range(2):
                nc.vector.tensor_mul(esq[k][:], eh2[k][:], eh2[k][:])
            pne = pmisc.tile([1, EB], F32, tag="pmisc", name="pne")
            for k in range(2):
                nc.tensor.matmul(
                    pne[:], ones_s[:], esq[k][:], start=(k == 0), stop=(k == 1)
                )
            ne2_s = smpool.tile([1, BC], F32)
            nc.scalar.copy(ne2_s[:], pne[:, 0:BC])
            pne2bc = pmisc.tile([128, BC], F32, tag="pmisc", name="pne2bc")
            nc.tensor.matmul(
                pne2bc[:], onesrow_s[:], ne2_s[:], start=True, stop=True
            )
            ne2bc_s = cpool.tile([128, BC], F32)
            nc.scalar.copy(ne2bc_s[:], pne2bc[:])

            # persistent SBUF accumulators, n on partitions, b on free
            traw_s = cpool.tile([128, BC], F32)
            ncsq_s = cpool.tile([128, BC], F32)

            # ---- context groups ----
            for g in range(GROUPS):
                xt = xtpool.tile([128, KT * 512], BF16)
                if fast:
                    # per subtile s: xt[p, s, jj, c*128+pbn] (s-major blocks)
                    for s in range(SUBT):
                        nc.gpsimd.dma_gather(
                            out_ap=xt[:]
                            .rearrange("p (z j i) -> p z j i", z=SUBT, j=3)[
                                :, s, :, :
                            ],
                            in_ap=ctab.ap(),
                            idxs_ap=cidx_s[
                                :, 32 * (SUBT * g + s) : 32 * (SUBT * g + s + 1)
                            ],
                            num_idxs=512,
                            num_idxs_reg=512,
                            elem_size=EP,
                            transpose=True,
                        )
                else:
                    xg = xgpool.tile([128, SUBT * CE], BF16)
                    nc.vector.memset(
                        xg[:].rearrange("p (q e) -> p q e", e=EP)[:, :, E:EP],
                        0.0,
                    )
                    for s in range(SUBT):
                        nc.gpsimd.indirect_dma_start(
                            out=xg[:]
                            .rearrange("p (q c e) -> p q c e", q=SUBT, c=C)[
                                :, s, :, 0:E
                            ],
                            out_offset=None,
                            in_=ctab.ap(),
                            in_offset=IndirectOffsetOnAxis(
                                ap=cidx_s[
                                    :, (SUBT * g + s) * C : (SUBT * g + s + 1) * C
                                ],
                                axis=0,
                            ),
                        )
                    for s in range(SUBT):
                        nc.sync.dma_start_transpose(
                            xt[:].rearrange(
                                "p (j z i) -> p j z i", j=KT, z=SUBT
                            )[:, :, s, :],
                            xg[:, CE * s : CE * (s + 1)],
                        )

                def xt_k(j):
                    # K-tile j = 3*c + jj; cols ordered (s, pbn)
                    if fast:
                        off = 512 * (j % 3) + 128 * (j // 3)
                        return xt[:].rearrange(
                            "p (z x) -> p z x", z=SUBT
                        )[:, :, off : off + 128]
                    return xt[:, 512 * j : 512 * (j + 1)]

                s1 = [
                    s1pool.tile([128, 512], BF16, tag=f"s1_{m}", name=f"s1_{m}")
                    for m in range(4)
                ]
                for m in range(4):
                    p1 = pm1.tile([128, 512], F32)
                    for j in range(KT):
                        nc.tensor.matmul(
                            p1[:],
                            w1t_s[:, H1 * j + 128 * m : H1 * j + 128 * m + 128],
                            xt_k(j),
                            start=(j == 0),
                            stop=(j == KT - 1),
                        )
                    nc.scalar.activation(
                        s1[m][:], p1[:], AF.Relu, bias=b1_s[:, m : m + 1]
                    )

                s2 = [
                    s2pool.tile([128, 512], BF16, tag=f"s2_{m}", name=f"s2_{m}")
                    for m in range(2)
                ]
                for m in range(2):
                    p2 = pm2.tile([128, 512], F32)
                    for j in range(4):
                        nc.tensor.matmul(
                            p2[:],
                            w2t_s[:, H2 * j + 128 * m : H2 * j + 128 * m + 128],
                            s1[j][:],
                            start=(j == 0),
                            stop=(j == 3),
                        )
                    nc.scalar.activation(
                        s2[m][:], p2[:], AF.Relu, bias=b2_s[:, m : m + 1]
                    )

                csq = [
                    csqpool.tile([128, 512], BF16, tag=f"csq_{m}", name=f"csq_{m}")
                    for m in range(2)
                ]
                for m in range(2):
                    nc.vector.tensor_mul(csq[m][:], s2[m][:], s2[m][:])

                # raw dots and |c|^2, n on partitions, one column per b
                pT = ptn.tile([128, SUBT], F32, tag="pT", name="pT")
                pN = ptn.tile([128, SUBT], F32, tag="pN", name="pN")
                for s in range(SUBT):
                    b = SUBT * g + s
                    for k in range(2):
                        nc.tensor.matmul(
                            pT[:, s : s + 1],
                            s2[k][:, 128 * s : 128 * (s + 1)],
                            eh2[k][:, b : b + 1],
                            start=(k == 0),
                            stop=(k == 1),
                        )
                    for k in range(2):
                        nc.tensor.matmul(
                            pN[:, s : s + 1],
                            csq[k][:, 128 * s : 128 * (s + 1)],
                            ones_s[:],
                            start=(k == 0),
                            stop=(k == 1),
                        )
                nc.scalar.copy(traw_s[:, SUBT * g : SUBT * (g + 1)], pT[:])
                nc.scalar.copy(ncsq_s[:, SUBT * g : SUBT * (g + 1)], pN[:])

            # ---- kernel pooling (tiles are [n=128, b=64]) ----
            prodn = smpool.tile([128, BC], F32, tag="prodn")
            nc.vector.tensor_mul(prodn[:], ncsq_s[:], ne2bc_s[:])
            lnp = smpool.tile([128, BC], F32, tag="lnp")
            nc.scalar.activation(lnp[:], prodn[:], AF.Ln, bias=eps_s[:])
            nrmf = smpool.tile([128, BC], F32, tag="nrmf")
            nc.scalar.activation(nrmf[:], lnp[:], AF.Exp, scale=-0.5)
            trans = cpool.tile([128, BC], F32)
            nc.vector.tensor_mul(trans[:], traw_s[:], nrmf[:])

            kpp_s = cpool.tile([1, NK * BC], F32)
            for k in range(NK):
                sq = smpool.tile([128, BC], F32, tag="sq", name="sq")
                nc.scalar.activation(
                    sq[:], trans[:], AF.Square, bias=mub_s[:, k : k + 1]
                )
                arg = smpool.tile([128, BC], F32, tag="arg", name="arg")
                nc.vector.tensor_scalar(
                    arg[:], sq[:],
                    -1.0 / (2.0 * SIGMAS[k] ** 2), -87.0,
                    mybir.AluOpType.mult, mybir.AluOpType.max,
                )
                ek = smpool.tile([128, BC], F32, tag="ek", name="ek")
                nc.scalar.activation(ek[:], arg[:], AF.Exp)
                pp = pmisc.tile([1, BC], F32, tag="pmisc", name="pp")
                nc.tensor.matmul(pp[:], onesf_s[:], ek[:], start=True, stop=True)
                nc.scalar.copy(kpp_s[:, BC * k : BC * (k + 1)], pp[:])

            kpc_s = smpool.tile([1, NK * BC], F32, tag="kpc")
            nc.vector.tensor_scalar_max(kpc_s[:], kpp_s[:], 1e-10)
            kpl_s = smpool.tile([1, NK * BC], F32, tag="kpl")
            nc.scalar.activation(kpl_s[:], kpc_s[:], AF.Ln)

            # weighted sum over k: kps[b] = sum_k wckp[k] * kpl[k, b]
            kpw_s = smpool.tile([1, BC * NK], F32, tag="kpw")
            kpl_v = kpl_s[:].rearrange("p (k b) -> p b k", k=NK)
            wck_v = wckp_s[:][:, None, :].broadcast_to([1, BC, NK])
            kpw_v = kpw_s[:].rearrange("p (b k) -> p b k", b=BC)
            nc.vector.tensor_tensor(
                out=kpw_v, in0=kpl_v, in1=wck_v, op=mybir.AluOpType.mult
            )
            kps_s = smpool.tile([1, BC], F32, tag="kps")
            nc.vector.reduce_sum(
                out=kps_s[:], in_=kpw_v, axis=mybir.AxisListType.X
            )

            # ---- final score ----
            psc = pmisc.tile([1, BC], F32, tag="pmisc", name="psc")
            nc.tensor.matmul(psc[:], wct_s[:], feat_s[:], start=True, stop=True)
            tot_s = smpool.tile([1, BC], F32, tag="tot")
            nc.vector.tensor_add(tot_s[:], psc[:], kps_s[:])
            emx = smpool.tile([1, BC], F32, tag="emx")
            nc.scalar.activation(emx[:], tot_s[:], AF.Exp, bias=bc_s[:], scale=-1.0)
            emx1 = smpool.tile([1, BC], F32, tag="emx1")
            nc.vector.tensor_scalar_add(emx1[:], emx[:], 1.0)
            outs = smpool.tile([1, BC], F32, tag="outs")
            nc.vector.reciprocal(outs[:], emx1[:])
            nc.sync.dma_start(out_d.ap().rearrange("b one -> one b"), outs[:])

    nc.compile()

    # Spread SWDGE gathers across the 4 queues. The ucode locks each DMASW
    # semaphore lane to one queue, and Tile assigns lanes round-robin in
    # scheduled order, so derive queue from the assigned lane post-compile.
    import re as _re
    for blk in nc.m.functions[0].blocks:
        for inst in blk.instructions:
            if type(inst).__name__ == "InstDMAGatherAnt":
                for u in inst.sync_info.on_update:
                    m = _re.match(r"DMASW(\d+)_", u.ant_name or "")
                    if m:
                        inst.queue_num = int(m.group(1)) % 4
                        break

    _PROGRAM_CACHE[fast] = nc
    return nc


def _wrap16(flat_idx):
    """int16 index list -> (128, n/16) tile layout: unwrapped[i] =
    tile[i % 16, i // 16], replicated into all 8 16-partition stripes."""
    n = flat_idx.shape[0]
    t = np.zeros((16, n // 16), np.int16)
    t[np.arange(n) % 16, np.arange(n) // 16] = flat_idx
    return np.tile(t, (8, 1))


def _prep_core_inputs(inputs, core, fast):
    """Host-side shard + weight re-layouts for one core."""
    W1 = np.asarray(inputs["W1"], np.float32)
    W2 = np.asarray(inputs["W2"], np.float32)
    Wv = np.asarray(inputs["Wv"], np.float32)
    Wc = np.asarray(inputs["Wc"], np.float32)
    b1 = np.asarray(inputs["b1"], np.float32)
    b2 = np.asarray(inputs["b2"], np.float32)
    bv = np.asarray(inputs["bv"], np.float32)
    bc = np.asarray(inputs["bc"], np.float32)

    sl = slice(core * BC, (core + 1) * BC)
    ev = np.asarray(inputs["batch_event"][sl], np.int64)          # (BC, C)
    feats = np.asarray(inputs["batch_features"][sl], np.float32)  # (BC, NF)
    dists = np.asarray(inputs["batch_distances"][sl], np.float32) # (BC, 9)
    ctx = np.asarray(inputs["batch_context"][sl], np.int64)       # (BC, N, C)

    bf = ml_dtypes.bfloat16
    # W1.T with K padded 300->EP per component, zeros in the pad rows
    w1t = np.zeros((CE, H1), np.float32)
    for c in range(C):
        w1t[EP * c : EP * c + E, :] = W1[:, E * c : E * (c + 1)].T
    wvt = np.zeros((CE, 9), np.float32)
    wvt[EP * 1 : EP * 1 + E, :] = Wv.T  # predicates = component 1

    wc_full = np.zeros((128,), np.float32)
    wc_full[32 : 32 + 9] = Wc[0, 0:9]          # dist_emb block
    wc_full[64 : 64 + NF] = Wc[0, 9 : 9 + NF]  # batch_features block
    wckp = (Wc[0, NF + 9 :] * 0.01).astype(np.float32)  # kp block, 0.01 folded

    m = {
        "w1t": w1t.astype(bf),
        "w2t": np.ascontiguousarray(W2.T).astype(bf),
        "wvt": wvt.astype(bf),
        "b1d": np.ascontiguousarray(b1.reshape(4, 128).T),
        "b2d": np.ascontiguousarray(b2.reshape(2, 128).T),
        "bvd": bv.reshape(9, 1),
        "wct": wc_full.reshape(-1, 1),
        "wckp": wckp.reshape(1, NK),
        "bcd": -bc.reshape(1, 1),
        "ndsq": np.ascontiguousarray(-(dists * dists).T),
        "featT": np.ascontiguousarray(feats.T),
    }

    if fast:
        table = np.asarray(inputs["event_table"])
        allidx = np.concatenate([ctx.reshape(-1), ev.reshape(-1)])
        uniq, inv = np.unique(allidx, return_inverse=True)
        assert len(uniq) <= CT
        ctab = np.zeros((CT, EP), bf)
        ctab[: len(uniq), :E] = np.asarray(table[uniq], np.float32)
        rctx = inv[: ctx.size].astype(np.int16).reshape(BC, N, C)
        rev = inv[ctx.size :].astype(np.int16).reshape(BC, C)

        # context: per (g, s) gather of 512 idx with i = c*128 + p
        ci = rctx.reshape(GROUPS, SUBT, N, C).transpose(0, 1, 3, 2)  # g,s,c,p
        cidx = np.concatenate(
            [
                _wrap16(ci[g, s].reshape(-1))
                for g in range(GROUPS)
                for s in range(SUBT)
            ],
            axis=1,
        )
        # event: i = c*128 + b; b >= BC -> row 0 junk
        ei = np.zeros((C, 128), np.int16)
        ei[:, :BC] = rev.T
        m["ctab"] = ctab
        m["cidx"] = np.ascontiguousarray(cidx)
        m["eidx"] = np.ascontiguousarray(_wrap16(ei.reshape(-1)))
    else:
        m["table"] = np.ascontiguousarray(
            np.asarray(inputs["event_table"], np.float32)
        )
        m["ctxidx"] = np.ascontiguousarray(
            ctx.astype(np.int32).transpose(1, 0, 2).reshape(128, BC * C)
        )
        m["evidx"] = ev.astype(np.int32)
    return m


def kernel(**inputs) -> np.ndarray:
    # fast path requires every shard's unique row count to fit int16
    fast = True
    ctx = np.asarray(inputs["batch_context"], np.int64)
    ev = np.asarray(inputs["batch_event"], np.int64)
    for core in range(NCORES):
        sl = slice(core * BC, (core + 1) * BC)
        nuniq = len(np.unique(np.concatenate(
            [ctx[sl].reshape(-1), ev[sl].reshape(-1)])))
        if nuniq > CT:
            fast = False
            break
    nc = _build_program(fast)
    in_maps = [_prep_core_inputs(inputs, core, fast) for core in range(NCORES)]
    res = run_bass_kernel_spmd(nc, in_maps, core_ids=list(range(NCORES)))
    return np.concatenate([r["out"] for r in res.results], axis=0)


if __name__ == "__main__":
    nc = _build_program(True)
    print("program built ok")



# revision 8
# speedup vs baseline: 1.3699x; 1.3699x over previous
"""Trainium2 Bass kernel for nn_EventPairCompositionModel.

Strategy (data-parallel over batch, 8 cores, B=512 -> 64 per core):
  - Host compacts the f32 table per core to the ~24K unique rows its shard
    touches.  Embeddings are stored fp8 (e4m3, x16 scaled): elems 0..255 in a
    256B-row table fetched on-device with SWDGE transpose dma_gather
    (row-rate-bound, so fp8 minimizes bytes); elems 256..299 ride in a small
    host-packed per-(b,n) tail table loaded with one plain contiguous DMA.
  - The 16-bit-granularity transpose gather lands fp8 element pairs
    (2p, 2p+1) in 16-bit cells -> exactly the [K=128, 2] operand layout the
    PE's fp8 DoubleRow perf mode wants: the shared arg-composition MLP runs
    at 2x bf16 throughput, 5 DoubleRow passes for the K=1200 first layer
    (4 component mains + 1 combined tail) with zero pad waste.
  - s1 activations are stored fp8 (scales folded into activation scale/bias)
    so MLP2 is DoubleRow too; s2/eh2 stay bf16.
  - Cosine numerators via 1-col stationary eh2-column matmuls, |c|^2 via
    ones-matmuls; [1,512] PSUM rows are assembled into [1,8192] SBUF rows and
    one DMA re-lands them [64b, 128n] so KNRM pooling is vector reduce_sum
    over the free axis and the final score math runs 64 lanes wide.
  - If a shard ever touches >32767 unique rows (can't happen for random
    inputs), falls back to the original bf16 indirect-DMA path.
All 8 cores run the identical program on their own batch shard (SPMD, no
collectives); host concatenates the 8 (64,1) outputs.
"""

import numpy as np
import ml_dtypes

import concourse.bacc as bacc
import concourse.bass as bass
import concourse.tile as tile
import concourse.mybir as mybir
from concourse.bass import IndirectOffsetOnAxis
from concourse.bass_utils import run_bass_kernel_spmd
from concourse import library_config

F32 = mybir.dt.float32
BF16 = mybir.dt.bfloat16
F8 = mybir.dt.float8e4
I16 = mybir.dt.int16
I32 = mybir.dt.int32
AF = mybir.ActivationFunctionType
DR = mybir.MatmulPerfMode.DoubleRow

# Problem shapes (hardcoded per spec)
B, N, C, E = 512, 128, 4, 300
V = 50000
H1, H2 = 512, 256
NF, NK = 8, 11
NCORES = 8
BC = B // NCORES          # 64 batches per core
EM = 256                  # fp8 main row elems (0..255)
ET = E - EM               # 44 tail elems per component
KT = C * ET               # 176 combined tail K-rows
CT = 32768                # compact table rows (int16-indexable)
GROUPS = (BC * N) // 512  # 16 groups of 512 (b,n) pairs
SUBT = 4                  # 128-bn subtiles per group (one batch b each)
BN = BC * N               # 8192 (b,n) pairs per core
XS = 16.0                 # fp8 table scale

MUS = [1.0, 0.9, 0.7, 0.5, 0.3, 0.1, -0.1, -0.3, -0.5, -0.7, -0.9]
SIGMAS = [1e-3] + [0.1] * 10

_PROGRAM_CACHE = {}


def _build_fast():
    if "fast" in _PROGRAM_CACHE:
        return _PROGRAM_CACHE["fast"]

    nc = bacc.Bacc("TRN2", target_bir_lowering=False, debug=False, num_swdge_queues=4)

    # ---- DRAM I/O ----
    ctab = nc.dram_tensor("ctab", (CT, EM), F8, kind="ExternalInput")
    tailc = nc.dram_tensor("tailc", (128, 2 * BN), F8, kind="ExternalInput")
    taile = nc.dram_tensor("taile", (128, 2 * 128), F8, kind="ExternalInput")
    cidx = nc.dram_tensor("cidx", (128, GROUPS * 128), I16, kind="ExternalInput")
    eidx = nc.dram_tensor("eidx", (128, 32), I16, kind="ExternalInput")
    # weights: K-order-matched fp8 layouts (see _prep_core_inputs)
    w1m = nc.dram_tensor("w1m", (128, C * 2 * H1), F8, kind="ExternalInput")
    w1t = nc.dram_tensor("w1t", (128, 2 * H1), F8, kind="ExternalInput")
    w2q = nc.dram_tensor("w2q", (128, 4 * H2), F8, kind="ExternalInput")
    wvm = nc.dram_tensor("wvm", (128, 2 * 16), F8, kind="ExternalInput")
    wvt = nc.dram_tensor("wvt", (128, 2 * 16), F8, kind="ExternalInput")
    b1d = nc.dram_tensor("b1d", (128, 4), F32, kind="ExternalInput")   # 8*b1
    b2d = nc.dram_tensor("b2d", (128, 2), F32, kind="ExternalInput")
    bvd = nc.dram_tensor("bvd", (9, 1), F32, kind="ExternalInput")
    wcb = nc.dram_tensor("wcb", (BC, 48), F32, kind="ExternalInput")   # per-b Wc row
    wkb = nc.dram_tensor("wkb", (BC, NK), F32, kind="ExternalInput")   # 0.01*Wc_kp
    bcd = nc.dram_tensor("bcd", (BC, 1), F32, kind="ExternalInput")    # -bc
    ndsq = nc.dram_tensor("ndsq", (9, BC), F32, kind="ExternalInput")  # -(d*d).T
    featb = nc.dram_tensor("featb", (BC, NF), F32, kind="ExternalInput")
    out_d = nc.dram_tensor("out", (BC, 1), F32, kind="ExternalOutput")

    with tile.TileContext(nc) as tc:
        with (
            tc.tile_pool(name="consts", bufs=1) as cpool,
            tc.tile_pool(name="xt", bufs=8) as xtpool,
            tc.tile_pool(name="s1", bufs=3) as s1pool,
            tc.tile_pool(name="s2", bufs=3) as s2pool,
            tc.tile_pool(name="csq", bufs=3) as csqpool,
            tc.tile_pool(name="small", bufs=2) as smpool,
            tc.tile_pool(name="pm1", bufs=2, space="PSUM") as pm1,
            tc.tile_pool(name="pm2", bufs=2, space="PSUM") as pm2,
            tc.tile_pool(name="ptn", bufs=1, space="PSUM") as ptn,
            tc.tile_pool(name="pmisc", bufs=2, space="PSUM") as pmisc,
            tc.tile_pool(name="dsc", bufs=1, space="DRAM") as dpool,
        ):
            nc.gpsimd.load_library(library_config.mlp)
            # ---- index loads first so gathers can start ASAP ----
            cidx_s = cpool.tile([128, GROUPS * 128], I16)
            nc.sync.dma_start(cidx_s[:], cidx.ap())
            eidx_s = cpool.tile([128, 32], I16)
            nc.sync.dma_start(eidx_s[:], eidx.ap())

            # ---- event gather (512 idx = (c, b): 64 real + 64 junk b) ----
            xe = cpool.tile([128, 2 * 512], F8)
            nc.gpsimd.dma_gather(
                out_ap=xe[:].rearrange("p (j i) -> p j i", j=2),
                in_ap=ctab.ap(),
                idxs_ap=eidx_s[:],
                num_idxs=512,
                num_idxs_reg=512,
                elem_size=EM,
                transpose=True,
            )

            # ---- context gathers (16 groups x 4 subtiles x 512 idx) ----
            xts = []
            for g in range(GROUPS):
                xt = xtpool.tile([128, SUBT * 2 * 512], F8, tag="xt", name=f"xt_{g}")
                for s in range(SUBT):
                    nc.gpsimd.dma_gather(
                        out_ap=xt[:].rearrange(
                            "p (z j i) -> p z j i", z=SUBT, j=2
                        )[:, s, :, :],
                        in_ap=ctab.ap(),
                        idxs_ap=cidx_s[
                            :, 32 * (SUBT * g + s) : 32 * (SUBT * g + s + 1)
                        ],
                        num_idxs=512,
                        num_idxs_reg=512,
                        elem_size=EM,
                        transpose=True,
                    )
                xts.append(xt)

            # ---- tail tables (plain contiguous DMA) ----
            tailc_s = cpool.tile([128, 2 * BN], F8)
            nc.sync.dma_start(tailc_s[:], tailc.ap())
            taile_s = cpool.tile([128, 2 * 128], F8)
            nc.scalar.dma_start(taile_s[:], taile.ap())

            # ---- weights / consts ----
            w1m_s = cpool.tile([128, C * 2 * H1], F8)
            nc.scalar.dma_start(w1m_s[:], w1m.ap())
            w1t_s = cpool.tile([128, 2 * H1], F8)
            nc.scalar.dma_start(w1t_s[:], w1t.ap())
            w2q_s = cpool.tile([128, 4 * H2], F8)
            nc.scalar.dma_start(w2q_s[:], w2q.ap())
            wvm_s = cpool.tile([128, 2 * 16], F8)
            nc.scalar.dma_start(wvm_s[:], wvm.ap())
            wvt_s = cpool.tile([128, 2 * 16], F8)
            nc.scalar.dma_start(wvt_s[:], wvt.ap())
            b1_s = cpool.tile([128, 4], F32)
            nc.sync.dma_start(b1_s[:], b1d.ap())
            b2_s = cpool.tile([128, 2], F32)
            nc.sync.dma_start(b2_s[:], b2d.ap())
            bv_s = cpool.tile([9, 1], F32)
            nc.sync.dma_start(bv_s[:], bvd.ap())
            wcb_s = cpool.tile([BC, 48], F32)
            nc.sync.dma_start(wcb_s[:], wcb.ap())
            wkb_s = cpool.tile([BC, NK], F32)
            nc.sync.dma_start(wkb_s[:], wkb.ap())
            bc_s = cpool.tile([BC, 1], F32)
            nc.sync.dma_start(bc_s[:], bcd.ap())
            ndsq_s = cpool.tile([9, BC], F32)
            nc.sync.dma_start(ndsq_s[:], ndsq.ap())
            featb_s = cpool.tile([BC, NF], F32)
            nc.sync.dma_start(featb_s[:], featb.ap())
            ones_s = cpool.tile([128, 1], BF16)
            nc.vector.memset(ones_s[:], 1.0)

            # DoubleRow helpers ------------------------------------------
            # gathered fp8 layout: byte (p, f) = elem 2p + f%2 of row idx[f//2]
            def xmain(t, c):
                # t: [128, 1024] fp8 (one subtile-block); component c cells
                return t.rearrange("p (i q) -> p q i", q=2)[:, :, 128 * c : 128 * (c + 1)]

            def w1_main(c, m):
                return w1m_s[:].rearrange("p (c q m) -> p c q m", c=C, q=2)[
                    :, c, :, 128 * m : 128 * (m + 1)
                ]

            def w1_tail(m):
                return w1t_s[:].rearrange("p (q m) -> p q m", q=2)[
                    :, :, 128 * m : 128 * (m + 1)
                ]

            # ---- event path (cols: 64 real b + 64 junk) ----
            s1e = cpool.tile([128, 4 * 128], F8)
            for m in range(4):
                pe = pmisc.tile([128, 128], F32, tag="pmisc", name="pe")
                for c in range(C):
                    nc.tensor.matmul(
                        pe[:], w1_main(c, m), xmain(xe[:], c),
                        start=(c == 0), stop=False, perf_mode=DR,
                    )
                nc.tensor.matmul(
                    pe[:], w1_tail(m),
                    taile_s[:].rearrange("p (q i) -> p q i", q=2),
                    start=False, stop=True, perf_mode=DR,
                )
                # s1' = 8*relu(h1) = relu(psum/2 + 8*b1)
                nc.scalar.activation(
                    s1e[:, 128 * m : 128 * (m + 1)], pe[:], AF.Relu,
                    bias=b1_s[:, m : m + 1], scale=0.5,
                )

            eh2 = [
                cpool.tile([128, 128], BF16, tag=f"eh2_{k}", name=f"eh2_{k}")
                for k in range(2)
            ]
            for m in range(2):
                pe2 = pmisc.tile([128, 128], F32, tag="pmisc", name="pe2")
                for j in range(2):
                    nc.tensor.matmul(
                        pe2[:],
                        w2q_s[:].rearrange("p (u m) -> p u m", u=4)[
                            :, 2 * j : 2 * j + 2, 128 * m : 128 * (m + 1)
                        ],
                        s1e[:].rearrange("p (u i) -> p u i", u=4)[:, 2 * j : 2 * j + 2, :],
                        start=(j == 0), stop=(j == 1), perf_mode=DR,
                    )
                nc.scalar.activation(
                    eh2[m][:], pe2[:], AF.Relu, bias=b2_s[:, m : m + 1], scale=0.125
                )

            # variances -> dist_emb (component 1; scale 1/16 folded)
            pv = pmisc.tile([16, 128], F32, tag="pmisc", name="pv")
            nc.tensor.matmul(
                pv[:],
                wvm_s[:].rearrange("p (q m) -> p q m", q=2),
                xmain(xe[:], 1),
                start=True, stop=False, perf_mode=DR,
            )
            nc.tensor.matmul(
                pv[:],
                wvt_s[:].rearrange("p (q m) -> p q m", q=2),
                taile_s[:].rearrange("p (q i) -> p q i", q=2),
                start=False, stop=True, perf_mode=DR,
            )
            ez_s = smpool.tile([9, 128], F32)
            nc.scalar.activation(ez_s[:], pv[0:9, :], AF.Exp, bias=bv_s[:], scale=1.0 / XS)
            ez1_s = smpool.tile([9, 128], F32)
            nc.vector.tensor_scalar_add(ez1_s[:], ez_s[:], 1.0)
            var_s = smpool.tile([9, 128], F32)
            nc.scalar.activation(var_s[:], ez1_s[:], AF.Ln)
            rv_s = smpool.tile([9, BC], F32)
            nc.vector.reciprocal(rv_s[:], var_s[:, 0:BC])
            q_s = smpool.tile([9, BC], F32)
            nc.vector.tensor_mul(q_s[:], ndsq_s[:], rv_s[:])
            # dist_emb into padded [32, 64] (rows 9..31 zero) for transposes
            qp_s = cpool.tile([32, BC], F32)
            nc.vector.memset(qp_s[:], 0.0)
            nc.scalar.activation(qp_s[0:9, :], q_s[:], AF.Exp)

            # |e|^2 row then transpose to [64, 1] via DVE 32-blocks
            esq = [
                smpool.tile([128, 128], BF16, tag=f"esq_{k}", name=f"esq_{k}")
                for k in range(2)
            ]
            for k in range(2):
                nc.vector.tensor_mul(esq[k][:], eh2[k][:], eh2[k][:])
            pne = pmisc.tile([1, 128], F32, tag="pmisc", name="pne")
            for k in range(2):
                nc.tensor.matmul(
                    pne[:], ones_s[:], esq[k][:], start=(k == 0), stop=(k == 1)
                )
            ne2p = cpool.tile([32, BC], F32)
            nc.vector.memset(ne2p[:], 0.0)
            nc.scalar.copy(ne2p[0:1, :], pne[:, 0:BC])
            ne2b = cpool.tile([BC, 32], F32)
            nc.vector.transpose(ne2b[0:32, :], ne2p[:, 0:32])
            nc.vector.transpose(ne2b[32:64, :], ne2p[:, 32:64])

            # dist_emb transpose -> [64, 32]
            qb_s = cpool.tile([BC, 32], F32)
            nc.vector.transpose(qb_s[0:32, :], qp_s[:, 0:32])
            nc.vector.transpose(qb_s[32:64, :], qp_s[:, 32:64])

            # ---- context groups ----
            trawr = cpool.tile([1, BN], F32)
            ncsqr = cpool.tile([1, BN], F32)
            for g in range(GROUPS):
                xt = xts[g]

                def xmain_g(c):
                    # [p, q, z, i]: component c cells of all 4 subtiles
                    return xt[:].rearrange(
                        "p (z i q) -> p q z i", z=SUBT, q=2
                    )[:, :, :, 128 * c : 128 * (c + 1)]

                s1 = s1pool.tile([128, 4 * 512], F8, tag="s1", name=f"s1_{g}")
                for m in range(4):
                    p1 = pm1.tile([128, 512], F32)
                    for c in range(C):
                        nc.tensor.matmul(
                            p1[:], w1_main(c, m), xmain_g(c),
                            start=(c == 0), stop=False, perf_mode=DR,
                        )
                    nc.tensor.matmul(
                        p1[:], w1_tail(m),
                        tailc_s[:].rearrange("p (q i) -> p q i", q=2)[
                            :, :, 512 * g : 512 * (g + 1)
                        ],
                        start=False, stop=True, perf_mode=DR,
                    )
                    nc.scalar.activation(
                        s1[:, 512 * m : 512 * (m + 1)], p1[:], AF.Relu,
                        bias=b1_s[:, m : m + 1], scale=0.5,
                    )

                s2 = [
                    s2pool.tile([128, 512], BF16, tag=f"s2_{m}", name=f"s2_{m}")
                    for m in range(2)
                ]
                for m in range(2):
                    p2 = pm2.tile([128, 512], F32)
                    for j in range(2):
                        nc.tensor.matmul(
                            p2[:],
                            w2q_s[:].rearrange("p (u m) -> p u m", u=4)[
                                :, 2 * j : 2 * j + 2, 128 * m : 128 * (m + 1)
                            ],
                            s1[:].rearrange("p (u i) -> p u i", u=4)[
                                :, 2 * j : 2 * j + 2, :
                            ],
                            start=(j == 0), stop=(j == 1), perf_mode=DR,
                        )
                    nc.scalar.activation(
                        s2[m][:], p2[:], AF.Relu, bias=b2_s[:, m : m + 1], scale=0.125
                    )

                csq = [
                    csqpool.tile([128, 512], BF16, tag=f"csq_{m}", name=f"csq_{m}")
                    for m in range(2)
                ]
                for m in range(2):
                    nc.vector.tensor_mul(csq[m][:], s2[m][:], s2[m][:])

                # numerators: stationary = eh2 column of batch b = 4g+z
                pT = ptn.tile([1, 512], F32, tag="pT", name="pT")
                pN = ptn.tile([1, 512], F32, tag="pN", name="pN")
                for z in range(SUBT):
                    b = SUBT * g + z
                    for k in range(2):
                        nc.tensor.matmul(
                            pT[:, 128 * z : 128 * (z + 1)],
                            eh2[k][:, b : b + 1],
                            s2[k][:, 128 * z : 128 * (z + 1)],
                            start=(k == 0), stop=(k == 1),
                        )
                for k in range(2):
                    nc.tensor.matmul(
                        pN[:], ones_s[:], csq[k][:], start=(k == 0), stop=(k == 1)
                    )
                nc.scalar.copy(trawr[:, 512 * g : 512 * (g + 1)], pT[:])
                nc.vector.tensor_copy(ncsqr[:, 512 * g : 512 * (g + 1)], pN[:])

            # ---- re-land [1, 8192] rows as [64b, 128n] via DRAM scratch ----
            trd = dpool.tile([1, BN], F32, name="trd")
            ncd = dpool.tile([1, BN], F32, name="ncd")
            nc.sync.dma_start(trd[:], trawr[:])
            nc.scalar.dma_start(ncd[:], ncsqr[:])
            traw64 = cpool.tile([BC, N], F32)
            ncsq64 = cpool.tile([BC, N], F32)
            nc.sync.dma_start(
                traw64[:], trd[:].rearrange("o (b n) -> (o b) n", b=BC)
            )
            nc.scalar.dma_start(
                ncsq64[:], ncd[:].rearrange("o (b n) -> (o b) n", b=BC)
            )

            # trans = traw * (ncsq*ne2 + eps)^-0.5
            prodn = smpool.tile([BC, N], F32, tag="prodn")
            nc.vector.tensor_tensor(
                out=prodn[:], in0=ncsq64[:],
                in1=ne2b[:, 0:1].broadcast_to([BC, N]),
                op=mybir.AluOpType.mult,
            )
            lnp = smpool.tile([BC, N], F32, tag="lnp")
            eps_s = cpool.tile([BC, 1], F32)
            nc.vector.memset(eps_s[:], 1e-20)
            nc.scalar.activation(lnp[:], prodn[:], AF.Ln, bias=eps_s[:])
            nrmf = smpool.tile([BC, N], F32, tag="nrmf")
            nc.scalar.activation(nrmf[:], lnp[:], AF.Exp, scale=-0.5)
            trans = smpool.tile([BC, N], F32, tag="trans")
            nc.vector.tensor_mul(trans[:], traw64[:], nrmf[:])

            # ---- KNRM pooling: [64, 128] -> kp [64, 11] ----
            kpl = smpool.tile([BC, NK], F32, tag="kpl")
            mub_s = cpool.tile([BC, NK], F32)
            for k in range(NK):
                nc.vector.memset(mub_s[:, k : k + 1], -MUS[k])
            for k in range(NK):
                sq = smpool.tile([BC, N], F32, tag="sq", name="sq")
                nc.scalar.activation(
                    sq[:], trans[:], AF.Square, bias=mub_s[:, k : k + 1]
                )
                arg = smpool.tile([BC, N], F32, tag="arg", name="arg")
                nc.vector.tensor_scalar(
                    arg[:], sq[:],
                    -1.0 / (2.0 * SIGMAS[k] ** 2), -87.0,
                    mybir.AluOpType.mult, mybir.AluOpType.max,
                )
                ek = smpool.tile([BC, N], F32, tag="ek", name="ek")
                nc.scalar.activation(ek[:], arg[:], AF.Exp)
                pooled = smpool.tile([BC, 1], F32, tag="pooled", name="pooled")
                nc.vector.reduce_sum(
                    out=pooled[:], in_=ek[:], axis=mybir.AxisListType.X
                )
                kpc = smpool.tile([BC, 1], F32, tag="kpc", name="kpc")
                nc.vector.tensor_scalar_max(kpc[:], pooled[:], 1e-10)
                nc.scalar.activation(kpl[:, k : k + 1], kpc[:], AF.Ln)

            # weighted kp sum (0.01*Wc folded into wkb)
            kpw = smpool.tile([BC, NK], F32, tag="kpw")
            nc.vector.tensor_mul(kpw[:], kpl[:], wkb_s[:])
            kps = smpool.tile([BC, 1], F32, tag="kps")
            nc.vector.reduce_sum(out=kps[:], in_=kpw[:], axis=mybir.AxisListType.X)

            # ---- feature score + sigmoid ----
            featall = cpool.tile([BC, 48], F32)
            nc.vector.memset(featall[:], 0.0)
            nc.vector.tensor_copy(featall[:, 0:NF], featb_s[:])
            nc.vector.tensor_copy(featall[:, 16:48], qb_s[:])
            fw = smpool.tile([BC, 48], F32, tag="fw")
            nc.vector.tensor_mul(fw[:], featall[:], wcb_s[:])
            fs = smpool.tile([BC, 1], F32, tag="fs")
            nc.vector.reduce_sum(out=fs[:], in_=fw[:], axis=mybir.AxisListType.X)
            tot = smpool.tile([BC, 1], F32, tag="tot")
            nc.vector.tensor_add(tot[:], fs[:], kps[:])
            emx = smpool.tile([BC, 1], F32, tag="emx")
            nc.scalar.activation(emx[:], tot[:], AF.Exp, bias=bc_s[:], scale=-1.0)
            emx1 = smpool.tile([BC, 1], F32, tag="emx1")
            nc.vector.tensor_scalar_add(emx1[:], emx[:], 1.0)
            outs = smpool.tile([BC, 1], F32, tag="outs")
            nc.vector.reciprocal(outs[:], emx1[:])
            nc.sync.dma_start(out_d.ap(), outs[:])

    nc.compile()

    # Spread SWDGE gathers across the 4 queues (ucode locks each DMASW
    # semaphore lane to one queue; lanes are assigned round-robin in
    # scheduled order, so derive queue from the assigned lane post-compile).
    import re as _re
    for blk in nc.m.functions[0].blocks:
        for inst in blk.instructions:
            if type(inst).__name__ == "InstDMAGatherAnt":
                for u in inst.sync_info.on_update:
                    m = _re.match(r"DMASW(\d+)_", u.ant_name or "")
                    if m:
                        inst.queue_num = int(m.group(1)) % 4
                        break

    _PROGRAM_CACHE["fast"] = nc
    return nc


def _wrap16(flat_idx):
    """int16 index list -> (128, n/16) tile layout: unwrapped[i] =
    tile[i % 16, i // 16], replicated into all 8 16-partition stripes."""
    n = flat_idx.shape[0]
    t = np.zeros((16, n // 16), np.int16)
    t[np.arange(n) % 16, np.arange(n) // 16] = flat_idx
    return np.tile(t, (8, 1))


def _pack_dr_k(mat, rows):
    """[K, N] -> [128, 2, N] fp8 DoubleRow K-pair layout, zero-padded."""
    k, n = mat.shape
    assert k <= rows <= 256
    out = np.zeros((256, n), np.float32)
    out[:k] = mat
    return np.ascontiguousarray(
        out.reshape(2, 128, n).transpose(1, 0, 2).reshape(128, 2 * n)
    )


def _prep_fast_consts(inputs):
    """Shared (core-independent) fp8 weight re-layouts."""
    f8 = ml_dtypes.float8_e4m3fn
    W1 = np.asarray(inputs["W1"], np.float32)   # (H1, C*E)
    W2 = np.asarray(inputs["W2"], np.float32)   # (H2, H1)
    Wv = np.asarray(inputs["Wv"], np.float32)   # (9, E)
    b1 = np.asarray(inputs["b1"], np.float32)
    b2 = np.asarray(inputs["b2"], np.float32)
    bv = np.asarray(inputs["bv"], np.float32)

    W1q = W1.astype(f8).astype(np.float32)      # quantize once, reuse
    Wvq = Wv.astype(f8).astype(np.float32)

    # main: w1m[p, c, q, m] = W1q[m, E*c + 2p+q]  (elems < 256)
    w1m = np.zeros((128, C, 2, H1), np.float32)
    for c in range(C):
        blk = W1q[:, E * c : E * c + EM]        # (H1, 256)
        w1m[:, c, :, :] = blk.T.reshape(128, 2, H1)
    # tail: k_t = c*44 + (e-256); w1t[p, q, m] = W1q[m, ktmap(q*128+p)]
    tail = np.zeros((256, H1), np.float32)
    for c in range(C):
        tail[ET * c : ET * (c + 1)] = W1q[:, E * c + EM : E * (c + 1)].T
    w1t = tail.reshape(2, 128, H1).transpose(1, 0, 2)

    # w2: [p, u, m] = W2q[m, 128u + p]
    W2q = W2.astype(f8).astype(np.float32)
    w2q = W2q.T.reshape(4, 128, H2).transpose(1, 0, 2)

    # wv main (component 1 elems < 256) and tail rows 44..87
    wvm_full = np.zeros((256, 16), np.float32)
    wvm_full[:, 0:9] = Wvq[:, :EM].T
    wvm = wvm_full.reshape(2, 128, 16).transpose(1, 0, 2)
    wvt_full = np.zeros((256, 16), np.float32)
    wvt_full[ET : 2 * ET, 0:9] = Wvq[:, EM:E].T
    wvt = wvt_full.reshape(2, 128, 16).transpose(1, 0, 2)

    return {
        "w1m": np.ascontiguousarray(w1m.reshape(128, C * 2 * H1)).astype(f8),
        "w1t": np.ascontiguousarray(w1t.reshape(128, 2 * H1)).astype(f8),
        "w2q": np.ascontiguousarray(w2q.reshape(128, 4 * H2)).astype(f8),
        "wvm": np.ascontiguousarray(wvm.reshape(128, 2 * 16)).astype(f8),
        "wvt": np.ascontiguousarray(wvt.reshape(128, 2 * 16)).astype(f8),
        "b1d": np.ascontiguousarray(8.0 * b1.reshape(4, 128).T),
        "b2d": np.ascontiguousarray(b2.reshape(2, 128).T),
        "bvd": bv.reshape(9, 1),
    }


def _prep_fast_core(inputs, consts, tableq, core):
    """Per-core shard prep for the fast fp8 path."""
    f8 = ml_dtypes.float8_e4m3fn
    Wc = np.asarray(inputs["Wc"], np.float32)
    bc = np.asarray(inputs["bc"], np.float32)

    sl = slice(core * BC, (core + 1) * BC)
    ev = np.asarray(inputs["batch_event"][sl], np.int64)          # (BC, C)
    feats = np.asarray(inputs["batch_features"][sl], np.float32)  # (BC, NF)
    dists = np.asarray(inputs["batch_distances"][sl], np.float32) # (BC, 9)
    ctx = np.asarray(inputs["batch_context"][sl], np.int64)       # (BC, N, C)

    allidx = np.concatenate([ctx.reshape(-1), ev.reshape(-1)])
    uniq, inv = np.unique(allidx, return_inverse=True)
    assert len(uniq) <= CT
    tq = tableq[uniq]                                   # (U, E) fp8
    ctab = np.zeros((CT, EM), f8)
    ctab[: len(uniq)] = tq[:, :EM]
    rctx = inv[: ctx.size].astype(np.int64).reshape(BC, N, C)
    rev = inv[ctx.size :].astype(np.int64).reshape(BC, C)

    # context gather idx: per (g, s): 512 idx with i = c*128 + n
    ci = rctx.reshape(GROUPS, SUBT, N, C).transpose(0, 1, 3, 2)  # g,s,c,n
    cidx = np.concatenate(
        [
            _wrap16(ci[g, s].reshape(-1).astype(np.int16))
            for g in range(GROUPS)
            for s in range(SUBT)
        ],
        axis=1,
    )
    # event idx: i = c*128 + b; b >= BC -> row 0 junk
    ei = np.zeros((C, 128), np.int16)
    ei[:, :BC] = rev.T.astype(np.int16)

    # tails: [k_t = c*44 + e', col = 128b + n] from the SAME quantized table
    tl = tq[:, EM:E].astype(np.float32)                 # (U, 44)
    tailc = tl[rctx]                                    # (BC, N, C, 44)
    tailc = tailc.transpose(2, 3, 0, 1).reshape(KT, BN)
    taile = tl[rev].transpose(1, 2, 0).reshape(KT, BC)  # (176, 64)
    taile = np.concatenate([taile, np.zeros((KT, 128 - BC), np.float32)], axis=1)

    # per-b Wc rows: cols 0..7 features, 16..24 dist_emb
    wc_row = np.zeros((48,), np.float32)
    wc_row[0:NF] = Wc[0, 9 : 9 + NF]
    wc_row[16 : 16 + 9] = Wc[0, 0:9]
    wkp = (Wc[0, NF + 9 :] * 0.01).astype(np.float32)

    m = dict(consts)
    m.update(
        {
            "ctab": ctab,
            "tailc": _pack_dr_k(tailc, KT).astype(f8),
            "taile": _pack_dr_k(taile, KT).astype(f8),
            "cidx": np.ascontiguousarray(cidx),
            "eidx": np.ascontiguousarray(_wrap16(ei.reshape(-1))),
            "wcb": np.tile(wc_row, (BC, 1)),
            "wkb": np.tile(wkp, (BC, 1)),
            "bcd": np.full((BC, 1), -float(bc[0]), np.float32),
            "ndsq": np.ascontiguousarray(-(dists * dists).T),
            "featb": np.ascontiguousarray(feats),
        }
    )
    return m


def _numpy_fallback(inputs):
    """Exact reference math on host (safety net for >32K unique rows)."""
    t = np.asarray(inputs["event_table"], np.float32)
    W1 = np.asarray(inputs["W1"], np.float32)
    b1 = np.asarray(inputs["b1"], np.float32)
    W2 = np.asarray(inputs["W2"], np.float32)
    b2 = np.asarray(inputs["b2"], np.float32)
    Wv = np.asarray(inputs["Wv"], np.float32)
    bv = np.asarray(inputs["bv"], np.float32)
    Wc = np.asarray(inputs["Wc"], np.float32)
    bc = np.asarray(inputs["bc"], np.float32)
    ev = np.asarray(inputs["batch_event"], np.int64)
    feats = np.asarray(inputs["batch_features"], np.float32)
    dists = np.asarray(inputs["batch_distances"], np.float32)
    ctx = np.asarray(inputs["batch_context"], np.int64)

    def mlp(x):
        x = np.maximum(x @ W1.T + b1, 0.0)
        return np.maximum(x @ W2.T + b2, 0.0)

    def l2n(x):
        n = np.linalg.norm(x, axis=-1, keepdims=True)
        return x / np.maximum(n, 1e-12)

    ee = t[ev]                                    # (B, C, E)
    ce = t[ctx]                                   # (B, N, C, E)
    var = np.log1p(np.exp(ee[:, 1, :] @ Wv.T + bv))
    de = np.exp(-(dists * dists) / var)
    extracted = np.concatenate([de, feats], axis=1)
    er = mlp(ee.reshape(B, 1, C * E))
    cr = mlp(ce.reshape(B, N, C * E))
    trans = np.einsum("bmd,bnd->bmn", l2n(er), l2n(cr))
    mus = np.array(MUS, np.float32)
    sig = np.array(SIGMAS, np.float32)
    kk = np.exp(-((trans[..., None] - mus) ** 2) / (2.0 * sig**2))
    kp = np.log(np.clip(kk.sum(axis=2), 1e-10, None)) * 0.01
    allf = np.concatenate([extracted[:, None, :], kp], axis=-1)
    scores = (allf @ Wc.T + bc)[..., 0]
    return (1.0 / (1.0 + np.exp(-scores))).astype(np.float32)


def kernel(**inputs) -> np.ndarray:
    ctx = np.asarray(inputs["batch_context"], np.int64)
    ev = np.asarray(inputs["batch_event"], np.int64)
    fast = True
    for core in range(NCORES):
        sl = slice(core * BC, (core + 1) * BC)
        nuniq = len(np.unique(np.concatenate(
            [ctx[sl].reshape(-1), ev[sl].reshape(-1)])))
        if nuniq > CT:
            fast = False
            break
    if not fast:  # pragma: no cover - impossible for random inputs
        return _numpy_fallback(inputs)

    f8 = ml_dtypes.float8_e4m3fn
    tableq = (np.asarray(inputs["event_table"], np.float32) * XS).astype(f8)
    consts = _prep_fast_consts(inputs)
    nc = _build_fast()
    in_maps = [
        _prep_fast_core(inputs, consts, tableq, core) for core in range(NCORES)
    ]
    res = run_bass_kernel_spmd(nc, in_maps, core_ids=list(range(NCORES)))
    return np.concatenate([r["out"] for r in res.results], axis=0)


if __name__ == "__main__":
    nc = _build_fast()
    print("program built ok")


# revision 20
# speedup vs baseline: 1.6360x; 1.1942x over previous
"""Trainium2 Bass kernel for nn_EventPairCompositionModel.

Strategy (data-parallel over batch, 8 cores, B=512 -> 64 per core):
  - Host compacts the f32 table per core to the ~24K unique rows its shard
    touches.  Embeddings are stored fp8 (e4m3, x16 scaled): elems 0..255 in a
    256B-row table fetched on-device with SWDGE transpose dma_gather
    (row-rate-bound, so fp8 minimizes bytes); elems 256..299 ride in a small
    host-packed per-(b,n) tail table loaded with one plain contiguous DMA.
  - The 16-bit-granularity transpose gather lands fp8 element pairs
    (2p, 2p+1) in 16-bit cells -> exactly the [K=128, 2] operand layout the
    PE's fp8 DoubleRow perf mode wants: the shared arg-composition MLP runs
    at 2x bf16 throughput, 5 DoubleRow passes for the K=1200 first layer
    (4 component mains + 1 combined tail) with zero pad waste.
  - All activations are fp8 with scales folded into activation scale/bias
    (s1 = 8*h1, s2 = 8*h2), so MLP2, cosine numerators (stationary = event
    column) and |c|^2 ones-reductions are DoubleRow as well.
  - relu1 runs on Scalar, relu2 on Vector (engine balance); per-group [1,512]
    numerator/norm rows go SBUF->DRAM scratch and are re-read [64b, 128n] so
    KNRM pooling is 4 wide vector ops + one batched Exp + one reduce_sum
    (no activation-table thrash), and the final score math runs 64 lanes wide
    with Softplus/Rsqrt/Sigmoid activations.
  - If a shard ever touches >32767 unique rows (can't happen for random
    inputs), falls back to exact host math.
All 8 cores run the identical program on their own batch shard (SPMD, no
collectives); host concatenates the 8 (64,1) outputs.
"""

import numpy as np
import ml_dtypes

import concourse.bacc as bacc
import concourse.bass as bass
import concourse.tile as tile
import concourse.mybir as mybir
from concourse.bass_utils import run_bass_kernel_spmd
from concourse import library_config

F32 = mybir.dt.float32
BF16 = mybir.dt.bfloat16
F8 = mybir.dt.float8e4
I16 = mybir.dt.int16
AF = mybir.ActivationFunctionType
DR = mybir.MatmulPerfMode.DoubleRow
MUL = mybir.AluOpType.mult
ADD = mybir.AluOpType.add
MAXOP = mybir.AluOpType.max

# Problem shapes (hardcoded per spec)
B, N, C, E = 512, 128, 4, 300
V = 50000
H1, H2 = 512, 256
NF, NK = 8, 11
NCORES = 8
BC = B // NCORES          # 64 batches per core
EM = 256                  # fp8 main row elems (0..255)
ET = E - EM               # 44 tail elems per component
KT = C * ET               # 176 combined tail K-rows
CT = 32768                # compact table rows (int16-indexable)
GROUPS = (BC * N) // 512  # 16 groups of 512 (b,n) pairs
SUBT = 4                  # 128-bn subtiles per group (one batch b each)
BN = BC * N               # 8192 (b,n) pairs per core
XS = 16.0                 # fp8 table scale
HS = 8.0                  # fp8 hidden-activation scale

MUS = [1.0, 0.9, 0.7, 0.5, 0.3, 0.1, -0.1, -0.3, -0.5, -0.7, -0.9]
SIGMAS = [1e-3] + [0.1] * 10

_PROGRAM_CACHE = {}


def _build_fast():
    if "fast" in _PROGRAM_CACHE:
        return _PROGRAM_CACHE["fast"]

    nc = bacc.Bacc("TRN2", target_bir_lowering=False, debug=False, num_swdge_queues=4)

    # ---- DRAM I/O ----
    ctab = nc.dram_tensor("ctab", (CT, EM), F8, kind="ExternalInput")
    tailc = nc.dram_tensor("tailc", (128, 2 * BN), F8, kind="ExternalInput")
    taile = nc.dram_tensor("taile", (128, 2 * 128), F8, kind="ExternalInput")
    cidx = nc.dram_tensor("cidx", (128, GROUPS * 128), I16, kind="ExternalInput")
    eidx = nc.dram_tensor("eidx", (128, 32), I16, kind="ExternalInput")
    # weights: K-order-matched fp8 layouts (see _prep_fast_consts)
    w1m = nc.dram_tensor("w1m", (128, C * 2 * H1), F8, kind="ExternalInput")
    w1t = nc.dram_tensor("w1t", (128, 2 * H1), F8, kind="ExternalInput")
    w2q = nc.dram_tensor("w2q", (128, 4 * H2), F8, kind="ExternalInput")
    wvm = nc.dram_tensor("wvm", (128, 2 * 16), F8, kind="ExternalInput")
    wvt = nc.dram_tensor("wvt", (128, 2 * 16), F8, kind="ExternalInput")
    b1d = nc.dram_tensor("b1d", (128, 4), F32, kind="ExternalInput")   # 8*b1
    b2d = nc.dram_tensor("b2d", (128, 2), F32, kind="ExternalInput")   # 8*b2
    bvd = nc.dram_tensor("bvd", (9, 1), F32, kind="ExternalInput")
    wcb = nc.dram_tensor("wcb", (BC, 48), F32, kind="ExternalInput")   # per-b Wc row
    wkb = nc.dram_tensor("wkb", (BC, NK), F32, kind="ExternalInput")   # 0.01*Wc_kp
    bcd = nc.dram_tensor("bcd", (BC, 1), F32, kind="ExternalInput")    # -bc
    ndsq = nc.dram_tensor("ndsq", (9, BC), F32, kind="ExternalInput")  # -(d*d).T
    featd = nc.dram_tensor("featd", (BC, NF), F32, kind="ExternalInput")
    mur = nc.dram_tensor("mur", (BC, NK * N), F32, kind="ExternalInput")   # -mu_k rep
    sgr = nc.dram_tensor("sgr", (BC, NK * N), F32, kind="ExternalInput")   # -1/(2s^2) rep
    out_d = nc.dram_tensor("out", (BC, 1), F32, kind="ExternalOutput")

    with tile.TileContext(nc) as tc:
        with (
            tc.tile_pool(name="consts", bufs=1) as cpool,
            tc.tile_pool(name="xt", bufs=8) as xtpool,
            tc.tile_pool(name="s1", bufs=3) as s1pool,
            tc.tile_pool(name="s2", bufs=3) as s2pool,
            tc.tile_pool(name="csq", bufs=3) as csqpool,
            tc.tile_pool(name="small", bufs=2) as smpool,
            tc.tile_pool(name="pm1", bufs=2, space="PSUM") as pm1,
            tc.tile_pool(name="pm2", bufs=2, space="PSUM") as pm2,
            tc.tile_pool(name="ptn", bufs=1, space="PSUM") as ptn,
            tc.tile_pool(name="pmisc", bufs=2, space="PSUM") as pmisc,
            tc.tile_pool(name="dsc", bufs=1, space="DRAM") as dpool,
        ):
            nc.gpsimd.load_library(library_config.mlp)
            # ---- index loads first so gathers can start ASAP ----
            cidx_s = cpool.tile([128, GROUPS * 128], I16)
            nc.sync.dma_start(cidx_s[:], cidx.ap())
            eidx_s = cpool.tile([128, 32], I16)
            nc.sync.dma_start(eidx_s[:], eidx.ap())

            # ---- event gather (512 idx = (c, b): 64 real + 64 junk b) ----
            xe = cpool.tile([128, 2 * 512], F8)
            nc.gpsimd.dma_gather(
                out_ap=xe[:].rearrange("p (j i) -> p j i", j=2),
                in_ap=ctab.ap(),
                idxs_ap=eidx_s[:],
                num_idxs=512,
                num_idxs_reg=512,
                elem_size=EM,
                transpose=True,
            )

            # ---- context gathers (16 groups x 4 subtiles x 512 idx) ----
            xts = []
            for g in range(GROUPS):
                xt = xtpool.tile([128, SUBT * 2 * 512], F8, tag="xt", name=f"xt_{g}")
                for s in range(SUBT):
                    nc.gpsimd.dma_gather(
                        out_ap=xt[:].rearrange(
                            "p (z j i) -> p z j i", z=SUBT, j=2
                        )[:, s, :, :],
                        in_ap=ctab.ap(),
                        idxs_ap=cidx_s[
                            :, 32 * (SUBT * g + s) : 32 * (SUBT * g + s + 1)
                        ],
                        num_idxs=512,
                        num_idxs_reg=512,
                        elem_size=EM,
                        transpose=True,
                    )
                xts.append(xt)

            # ---- tail tables (plain contiguous DMA) ----
            tailc_s = cpool.tile([128, 2 * BN], F8)
            nc.sync.dma_start(tailc_s[:], tailc.ap())
            taile_s = cpool.tile([128, 2 * 128], F8)
            nc.scalar.dma_start(taile_s[:], taile.ap())

            # ---- weights / consts ----
            w1m_s = cpool.tile([128, C * 2 * H1], F8)
            nc.scalar.dma_start(w1m_s[:], w1m.ap())
            w1t_s = cpool.tile([128, 2 * H1], F8)
            nc.scalar.dma_start(w1t_s[:], w1t.ap())
            w2q_s = cpool.tile([128, 4 * H2], F8)
            nc.scalar.dma_start(w2q_s[:], w2q.ap())
            wvm_s = cpool.tile([128, 2 * 16], F8)
            nc.scalar.dma_start(wvm_s[:], wvm.ap())
            wvt_s = cpool.tile([128, 2 * 16], F8)
            nc.scalar.dma_start(wvt_s[:], wvt.ap())
            b1_s = cpool.tile([128, 4], F32)
            nc.sync.dma_start(b1_s[:], b1d.ap())
            b2_s = cpool.tile([128, 2], F32)
            nc.sync.dma_start(b2_s[:], b2d.ap())
            bv_s = cpool.tile([9, 1], F32)
            nc.sync.dma_start(bv_s[:], bvd.ap())
            wcb_s = cpool.tile([BC, 48], F32)
            nc.sync.dma_start(wcb_s[:], wcb.ap())
            wkb_s = cpool.tile([BC, NK], F32)
            nc.sync.dma_start(wkb_s[:], wkb.ap())
            bc_s = cpool.tile([BC, 1], F32)
            nc.sync.dma_start(bc_s[:], bcd.ap())
            ndsq_s = cpool.tile([9, BC], F32)
            nc.sync.dma_start(ndsq_s[:], ndsq.ap())
            featd_s = cpool.tile([BC, NF], F32)
            nc.sync.dma_start(featd_s[:], featd.ap())
            mur_s = cpool.tile([BC, NK * N], F32)
            nc.sync.dma_start(mur_s[:], mur.ap())
            sgr_s = cpool.tile([BC, NK * N], F32)
            nc.sync.dma_start(sgr_s[:], sgr.ap())
            ones8 = cpool.tile([128, 32], F8)
            nc.vector.memset(ones8[:], 1.0)

            # DRAM scratch for numerator/norm rows
            trd = dpool.tile([1, BN], F32, name="trd")
            ncd = dpool.tile([1, BN], F32, name="ncd")

            # DoubleRow helpers ------------------------------------------
            # gathered fp8 layout: byte (p, f) = elem 2p + f%2 of row idx[f//2]
            def xmain(t, c):
                return t.rearrange("p (i q) -> p q i", q=2)[:, :, 128 * c : 128 * (c + 1)]

            def w1_main(c, m):
                return w1m_s[:].rearrange("p (c q m) -> p c q m", c=C, q=2)[
                    :, c, :, 128 * m : 128 * (m + 1)
                ]

            def w1_tail(m):
                return w1t_s[:].rearrange("p (q m) -> p q m", q=2)[
                    :, :, 128 * m : 128 * (m + 1)
                ]

            def relu2_vec(out8, psum, m, w=512):
                # s2' = relu(psum + 8*b2) in fp8 (scales folded)
                t = smpool.tile([128, 512], F32, tag="r2t", name="r2t")
                nc.vector.scalar_tensor_tensor(
                    out=t[:, 0:w], in0=psum, scalar=1.0,
                    in1=b2_s[:, m : m + 1].broadcast_to([128, w]),
                    op0=MUL, op1=ADD,
                )
                nc.vector.tensor_scalar_max(out8, t[:, 0:w], 0.0)

            # ---- event path (cols: 64 real b + 64 junk) ----
            s1e = cpool.tile([128, 4 * 128], F8)
            for m in range(4):
                pe = pmisc.tile([128, 128], F32, tag="pmisc", name="pe")
                for c in range(C):
                    nc.tensor.matmul(
                        pe[:], w1_main(c, m), xmain(xe[:], c),
                        start=(c == 0), stop=False, perf_mode=DR,
                    )
                nc.tensor.matmul(
                    pe[:], w1_tail(m),
                    taile_s[:].rearrange("p (q i) -> p q i", q=2),
                    start=False, stop=True, perf_mode=DR,
                )
                # s1' = 8*relu(h1) = relu(psum/2 + 8*b1)
                nc.scalar.activation(
                    s1e[:, 128 * m : 128 * (m + 1)], pe[:], AF.Relu,
                    bias=b1_s[:, m : m + 1], scale=0.5,
                )

            eh28 = cpool.tile([128, 2 * 128], F8)
            for m in range(2):
                pe2 = pmisc.tile([128, 128], F32, tag="pmisc", name="pe2")
                for j in range(2):
                    nc.tensor.matmul(
                        pe2[:],
                        w2q_s[:].rearrange("p (u m) -> p u m", u=4)[
                            :, 2 * j : 2 * j + 2, 128 * m : 128 * (m + 1)
                        ],
                        s1e[:].rearrange("p (u i) -> p u i", u=4)[:, 2 * j : 2 * j + 2, :],
                        start=(j == 0), stop=(j == 1), perf_mode=DR,
                    )
                relu2_vec(eh28[:, 128 * m : 128 * (m + 1)], pe2[:], m, w=128)

            # variance pre-activation (component 1); chain runs in end phase
            pv = pmisc.tile([16, 128], F32, tag="pmisc", name="pv")
            nc.tensor.matmul(
                pv[:],
                wvm_s[:].rearrange("p (q m) -> p q m", q=2),
                xmain(xe[:], 1),
                start=True, stop=False, perf_mode=DR,
            )
            nc.tensor.matmul(
                pv[:],
                wvt_s[:].rearrange("p (q m) -> p q m", q=2),
                taile_s[:].rearrange("p (q i) -> p q i", q=2),
                start=False, stop=True, perf_mode=DR,
            )
            pvs = cpool.tile([9, BC], F32)
            nc.vector.tensor_copy(pvs[:], pv[0:9, 0:BC])

            # |e|^2 row
            esq8 = cpool.tile([128, 2 * 128], F8)
            nc.vector.tensor_mul(esq8[:], eh28[:], eh28[:])
            pne = pmisc.tile([16, 128], F32, tag="pmisc", name="pne")
            nc.tensor.matmul(
                pne[:], ones8[:].rearrange("p (q m) -> p q m", q=2),
                esq8[:].rearrange("p (u i) -> p u i", u=2),
                start=True, stop=True, perf_mode=DR,
            )
            ne2p = cpool.tile([32, BC], F32)
            nc.vector.memset(ne2p[:], 0.0)
            nc.scalar.copy(ne2p[0:1, :], pne[0:1, 0:BC])
            ne2b = cpool.tile([BC, 32], F32)
            nc.vector.transpose(ne2b[0:32, :], ne2p[:, 0:32])
            nc.vector.transpose(ne2b[32:64, :], ne2p[:, 32:64])

            # ---- context groups ----
            for g in range(GROUPS):
                xt = xts[g]

                def xmain_g(c):
                    return xt[:].rearrange(
                        "p (z i q) -> p q z i", z=SUBT, q=2
                    )[:, :, :, 128 * c : 128 * (c + 1)]

                s1 = s1pool.tile([128, 4 * 512], F8, tag="s1", name=f"s1_{g}")
                for m in range(4):
                    p1 = pm1.tile([128, 512], F32)
                    for c in range(C):
                        nc.tensor.matmul(
                            p1[:], w1_main(c, m), xmain_g(c),
                            start=(c == 0), stop=False, perf_mode=DR,
                        )
                    nc.tensor.matmul(
                        p1[:], w1_tail(m),
                        tailc_s[:].rearrange("p (q i) -> p q i", q=2)[
                            :, :, 512 * g : 512 * (g + 1)
                        ],
                        start=False, stop=True, perf_mode=DR,
                    )
                    nc.scalar.activation(
                        s1[:, 512 * m : 512 * (m + 1)], p1[:], AF.Relu,
                        bias=b1_s[:, m : m + 1], scale=0.5,
                    )

                s28 = s2pool.tile([128, 2 * 512], F8, tag="s28", name=f"s28_{g}")
                for m in range(2):
                    p2 = pm2.tile([128, 512], F32)
                    for j in range(2):
                        nc.tensor.matmul(
                            p2[:],
                            w2q_s[:].rearrange("p (u m) -> p u m", u=4)[
                                :, 2 * j : 2 * j + 2, 128 * m : 128 * (m + 1)
                            ],
                            s1[:].rearrange("p (u i) -> p u i", u=4)[
                                :, 2 * j : 2 * j + 2, :
                            ],
                            start=(j == 0), stop=(j == 1), perf_mode=DR,
                        )
                    relu2_vec(s28[:, 512 * m : 512 * (m + 1)], p2[:], m)

                csq8 = csqpool.tile([128, 2 * 512], F8, tag="csq8", name=f"csq8_{g}")
                nc.vector.tensor_mul(csq8[:], s28[:], s28[:])

                s28v = s28[:].rearrange("p (u i) -> p u i", u=2)
                pT = ptn.tile([16, 512], F32, tag="pT", name="pT")
                pN = ptn.tile([16, 512], F32, tag="pN", name="pN")
                for z in range(SUBT):
                    b = SUBT * g + z
                    nc.tensor.matmul(
                        pT[0:16, 128 * z : 128 * (z + 1)],
                        eh28[:].rearrange("p (u i) -> p u i", u=2)[:, :, b : b + 16],
                        s28v[:, :, 128 * z : 128 * (z + 1)],
                        start=True, stop=True, perf_mode=DR,
                    )
                nc.tensor.matmul(
                    pN[:], ones8[:].rearrange("p (q m) -> p q m", q=2),
                    csq8[:].rearrange("p (u i) -> p u i", u=2),
                    start=True, stop=True, perf_mode=DR,
                )
                trow = smpool.tile([1, 512], F32, tag="trow", name=f"trow_{g}")
                nc.scalar.copy(trow[:], pT[0:1, :])
                nrow = smpool.tile([1, 512], F32, tag="nrow", name=f"nrow_{g}")
                nc.vector.tensor_copy(nrow[:], pN[0:1, :])
                nc.sync.dma_start(trd[:, 512 * g : 512 * (g + 1)], trow[:])
                nc.scalar.dma_start(ncd[:, 512 * g : 512 * (g + 1)], nrow[:])

            # ---- end phase: re-land [64b, 128n] ----
            traw64 = cpool.tile([BC, N], F32)
            ncsq64 = cpool.tile([BC, N], F32)
            nc.sync.dma_start(
                traw64[:], trd[:].rearrange("o (b n) -> (o b) n", b=BC)
            )
            nc.scalar.dma_start(
                ncsq64[:], ncd[:].rearrange("o (b n) -> (o b) n", b=BC)
            )

            # variance chain: var = softplus(pv/16 + bv); dist = exp(ndsq/var)
            ez_s = smpool.tile([9, BC], F32, tag="ez")
            nc.scalar.activation(ez_s[:], pvs[:], AF.Exp, bias=bv_s[:], scale=1.0 / XS)
            ez1_s = smpool.tile([9, BC], F32, tag="ez1")
            nc.vector.tensor_scalar_add(ez1_s[:], ez_s[:], 1.0)
            varb = smpool.tile([9, BC], F32, tag="varb")
            nc.scalar.activation(varb[:], ez1_s[:], AF.Ln)
            rv_s = smpool.tile([9, BC], F32, tag="rv")
            nc.vector.reciprocal(rv_s[:], varb[:])
            q_s = smpool.tile([9, BC], F32, tag="q")
            nc.vector.tensor_mul(q_s[:], ndsq_s[:], rv_s[:])

            # trans = traw * rsqrt(ncsq*ne2 + eps)
            prodn = smpool.tile([BC, N], F32, tag="prodn")
            nc.vector.tensor_tensor(
                out=prodn[:], in0=ncsq64[:],
                in1=ne2b[:, 0:1].broadcast_to([BC, N]),
                op=MUL,
            )
            eps_s = cpool.tile([BC, 1], F32)
            nc.vector.memset(eps_s[:], 1e-20)
            lnp = smpool.tile([BC, N], F32, tag="lnp")
            nc.scalar.activation(lnp[:], prodn[:], AF.Ln, bias=eps_s[:])
            nrmf = smpool.tile([BC, N], F32, tag="nrmf")
            nc.scalar.activation(nrmf[:], lnp[:], AF.Exp, scale=-0.5)
            trans = smpool.tile([BC, N], F32, tag="trans")
            nc.vector.tensor_mul(trans[:], traw64[:], nrmf[:])

            # ---- KNRM pooling, batched over all 11 kernels ----
            dk = smpool.tile([BC, NK * N], F32, tag="dk")
            nc.vector.tensor_tensor(
                out=dk[:].rearrange("b (k n) -> b k n", k=NK),
                in0=mur_s[:].rearrange("b (k n) -> b k n", k=NK),
                in1=trans[:, None, :].broadcast_to([BC, NK, N]),
                op=ADD,
            )
            sqk = smpool.tile([BC, NK * N], F32, tag="sqk")
            nc.vector.tensor_mul(sqk[:], dk[:], dk[:])
            argk = smpool.tile([BC, NK * N], F32, tag="argk")
            nc.vector.tensor_mul(argk[:], sqk[:], sgr_s[:])
            argc = smpool.tile([BC, NK * N], F32, tag="argc")
            nc.vector.tensor_scalar_max(argc[:], argk[:], -87.0)
            ekb = smpool.tile([BC, NK * N], F32, tag="ekb")
            # dist_emb exp shares the Exp table with the kernel exp
            qp_s = cpool.tile([32, BC], F32)
            nc.vector.memset(qp_s[:], 0.0)
            nc.scalar.activation(qp_s[0:9, :], q_s[:], AF.Exp)
            nc.scalar.activation(ekb[:], argc[:], AF.Exp)
            qb_s = cpool.tile([BC, 32], F32)
            nc.vector.transpose(qb_s[0:32, :], qp_s[:, 0:32])
            nc.vector.transpose(qb_s[32:64, :], qp_s[:, 32:64])
            pooled = smpool.tile([BC, NK], F32, tag="pooled")
            nc.vector.reduce_sum(
                out=pooled[:], in_=ekb[:].rearrange("b (k n) -> b k n", k=NK),
                axis=mybir.AxisListType.X,
            )
            kpc = smpool.tile([BC, NK], F32, tag="kpc")
            nc.vector.tensor_scalar_max(kpc[:], pooled[:], 1e-10)
            kpl = smpool.tile([BC, NK], F32, tag="kpl")
            nc.scalar.activation(kpl[:], kpc[:], AF.Ln)
            kpw = smpool.tile([BC, NK], F32, tag="kpw")
            nc.vector.tensor_mul(kpw[:], kpl[:], wkb_s[:])
            kps = smpool.tile([BC, 1], F32, tag="kps")
            nc.vector.reduce_sum(out=kps[:], in_=kpw[:], axis=mybir.AxisListType.X)

            # ---- feature score + sigmoid ----
            featall = cpool.tile([BC, 48], F32)
            nc.vector.memset(featall[:], 0.0)
            nc.vector.tensor_copy(featall[:, 0:NF], featd_s[:])
            nc.vector.tensor_copy(featall[:, 16:48], qb_s[:])
            fw = smpool.tile([BC, 48], F32, tag="fw")
            nc.vector.tensor_mul(fw[:], featall[:], wcb_s[:])
            fs = smpool.tile([BC, 1], F32, tag="fs")
            nc.vector.reduce_sum(out=fs[:], in_=fw[:], axis=mybir.AxisListType.X)
            tot = smpool.tile([BC, 1], F32, tag="tot")
            nc.vector.tensor_add(tot[:], fs[:], kps[:])
            emx = smpool.tile([BC, 1], F32, tag="emx")
            nc.scalar.activation(emx[:], tot[:], AF.Exp, bias=bc_s[:], scale=-1.0)
            emx1 = smpool.tile([BC, 1], F32, tag="emx1")
            nc.vector.tensor_scalar_add(emx1[:], emx[:], 1.0)
            outs = smpool.tile([BC, 1], F32, tag="outs")
            nc.vector.reciprocal(outs[:], emx1[:])
            nc.sync.dma_start(out_d.ap(), outs[:])

    nc.compile()

    # Spread SWDGE gathers across the 4 queues (ucode locks each DMASW
    # semaphore lane to one queue; lanes are assigned round-robin in
    # scheduled order, so derive queue from the assigned lane post-compile).
    import re as _re
    for blk in nc.m.functions[0].blocks:
        for inst in blk.instructions:
            if type(inst).__name__ == "InstDMAGatherAnt":
                for u in inst.sync_info.on_update:
                    m = _re.match(r"DMASW(\d+)_", u.ant_name or "")
                    if m:
                        inst.queue_num = int(m.group(1)) % 4
                        break

    _PROGRAM_CACHE["fast"] = nc
    return nc


def _wrap16(flat_idx):
    """int16 index list -> (128, n/16) tile layout: unwrapped[i] =
    tile[i % 16, i // 16], replicated into all 8 16-partition stripes."""
    n = flat_idx.shape[0]
    t = np.zeros((16, n // 16), np.int16)
    t[np.arange(n) % 16, np.arange(n) // 16] = flat_idx
    return np.tile(t, (8, 1))


def _pack_dr_k(mat, rows):
    """[K, N] -> [128, 2, N] fp8 DoubleRow K-pair layout, zero-padded."""
    k, n = mat.shape
    assert k <= rows <= 256
    out = np.zeros((256, n), np.float32)
    out[:k] = mat
    return np.ascontiguousarray(
        out.reshape(2, 128, n).transpose(1, 0, 2).reshape(128, 2 * n)
    )


def _prep_fast_consts(inputs):
    """Shared (core-independent) fp8 weight re-layouts."""
    f8 = ml_dtypes.float8_e4m3fn
    W1 = np.asarray(inputs["W1"], np.float32)   # (H1, C*E)
    W2 = np.asarray(inputs["W2"], np.float32)   # (H2, H1)
    Wv = np.asarray(inputs["Wv"], np.float32)   # (9, E)
    b1 = np.asarray(inputs["b1"], np.float32)
    b2 = np.asarray(inputs["b2"], np.float32)
    bv = np.asarray(inputs["bv"], np.float32)

    W1q = W1.astype(f8).astype(np.float32)      # quantize once, reuse
    Wvq = Wv.astype(f8).astype(np.float32)

    # main: w1m[p, c, q, m] = W1q[m, E*c + 2p+q]  (elems < 256)
    w1m = np.zeros((128, C, 2, H1), np.float32)
    for c in range(C):
        blk = W1q[:, E * c : E * c + EM]        # (H1, 256)
        w1m[:, c, :, :] = blk.T.reshape(128, 2, H1)
    # tail: k_t = c*44 + (e-256); w1t[p, q, m] = W1q[m, ktmap(q*128+p)]
    tail = np.zeros((256, H1), np.float32)
    for c in range(C):
        tail[ET * c : ET * (c + 1)] = W1q[:, E * c + EM : E * (c + 1)].T
    w1t = tail.reshape(2, 128, H1).transpose(1, 0, 2)

    # w2: [p, u, m] = W2q[m, 128u + p]
    W2q = W2.astype(f8).astype(np.float32)
    w2q = W2q.T.reshape(4, 128, H2).transpose(1, 0, 2)

    # wv main (component 1 elems < 256) and tail rows 44..87; M padded to 16
    wvm_full = np.zeros((256, 16), np.float32)
    wvm_full[:, 0:9] = Wvq[:, :EM].T
    wvm = wvm_full.reshape(2, 128, 16).transpose(1, 0, 2)
    wvt_full = np.zeros((256, 16), np.float32)
    wvt_full[ET : 2 * ET, 0:9] = Wvq[:, EM:E].T
    wvt = wvt_full.reshape(2, 128, 16).transpose(1, 0, 2)

    mus = np.array(MUS, np.float32)
    sig = np.array(SIGMAS, np.float32)
    mur = np.tile(np.repeat(-mus, N)[None, :], (BC, 1))
    sgr = np.tile(np.repeat(-1.0 / (2.0 * sig * sig), N)[None, :], (BC, 1))

    return {
        "w1m": np.ascontiguousarray(w1m.reshape(128, C * 2 * H1)).astype(f8),
        "w1t": np.ascontiguousarray(w1t.reshape(128, 2 * H1)).astype(f8),
        "w2q": np.ascontiguousarray(w2q.reshape(128, 4 * H2)).astype(f8),
        "wvm": np.ascontiguousarray(wvm.reshape(128, 2 * 16)).astype(f8),
        "wvt": np.ascontiguousarray(wvt.reshape(128, 2 * 16)).astype(f8),
        "b1d": np.ascontiguousarray(8.0 * b1.reshape(4, 128).T),
        "b2d": np.ascontiguousarray(8.0 * b2.reshape(2, 128).T),
        "bvd": bv.reshape(9, 1),
        "mur": np.ascontiguousarray(mur),
        "sgr": np.ascontiguousarray(sgr),
    }


def _prep_fast_core(inputs, consts, tableq, core):
    """Per-core shard prep for the fast fp8 path."""
    f8 = ml_dtypes.float8_e4m3fn
    Wc = np.asarray(inputs["Wc"], np.float32)
    bc = np.asarray(inputs["bc"], np.float32)

    sl = slice(core * BC, (core + 1) * BC)
    ev = np.asarray(inputs["batch_event"][sl], np.int64)          # (BC, C)
    feats = np.asarray(inputs["batch_features"][sl], np.float32)  # (BC, NF)
    dists = np.asarray(inputs["batch_distances"][sl], np.float32) # (BC, 9)
    ctx = np.asarray(inputs["batch_context"][sl], np.int64)       # (BC, N, C)

    allidx = np.concatenate([ctx.reshape(-1), ev.reshape(-1)])
    uniq, inv = np.unique(allidx, return_inverse=True)
    assert len(uniq) <= CT
    tq = tableq[uniq]                                   # (U, E) fp8
    ctab = np.zeros((CT, EM), f8)
    ctab[: len(uniq)] = tq[:, :EM]
    rctx = inv[: ctx.size].astype(np.int64).reshape(BC, N, C)
    rev = inv[ctx.size :].astype(np.int64).reshape(BC, C)

    # context gather idx: per (g, s): 512 idx with i = c*128 + n
    ci = rctx.reshape(GROUPS, SUBT, N, C).transpose(0, 1, 3, 2)  # g,s,c,n
    cidx = np.concatenate(
        [
            _wrap16(ci[g, s].reshape(-1).astype(np.int16))
            for g in range(GROUPS)
            for s in range(SUBT)
        ],
        axis=1,
    )
    # event idx: i = c*128 + b; b >= BC -> row 0 junk
    ei = np.zeros((C, 128), np.int16)
    ei[:, :BC] = rev.T.astype(np.int16)

    # tails: [k_t = c*44 + e', col = 128b + n] from the SAME quantized table
    tl = tq[:, EM:E].astype(np.float32)                 # (U, 44)
    tailc = tl[rctx]                                    # (BC, N, C, 44)
    tailc = tailc.transpose(2, 3, 0, 1).reshape(KT, BN)
    taile = tl[rev].transpose(1, 2, 0).reshape(KT, BC)  # (176, 64)
    taile = np.concatenate([taile, np.zeros((KT, 128 - BC), np.float32)], axis=1)

    # per-b Wc rows: cols 0..7 features, 16..24 dist_emb
    wc_row = np.zeros((48,), np.float32)
    wc_row[0:NF] = Wc[0, 9 : 9 + NF]
    wc_row[16 : 16 + 9] = Wc[0, 0:9]
    wkp = (Wc[0, NF + 9 :] * 0.01).astype(np.float32)

    m = dict(consts)
    m.update(
        {
            "ctab": ctab,
            "tailc": _pack_dr_k(tailc, KT).astype(f8),
            "taile": _pack_dr_k(taile, KT).astype(f8),
            "cidx": np.ascontiguousarray(cidx),
            "eidx": np.ascontiguousarray(_wrap16(ei.reshape(-1))),
            "wcb": np.tile(wc_row, (BC, 1)),
            "wkb": np.tile(wkp, (BC, 1)),
            "bcd": np.full((BC, 1), -float(bc[0]), np.float32),
            "ndsq": np.ascontiguousarray(-(dists * dists).T),
            "featd": np.ascontiguousarray(feats),
        }
    )
    return m


def _numpy_fallback(inputs):
    """Exact reference math on host (safety net for >32K unique rows)."""
    t = np.asarray(inputs["event_table"], np.float32)
    W1 = np.asarray(inputs["W1"], np.float32)
    b1 = np.asarray(inputs["b1"], np.float32)
    W2 = np.asarray(inputs["W2"], np.float32)
    b2 = np.asarray(inputs["b2"], np.float32)
    Wv = np.asarray(inputs["Wv"], np.float32)
    bv = np.asarray(inputs["bv"], np.float32)
    Wc = np.asarray(inputs["Wc"], np.float32)
    bc = np.asarray(inputs["bc"], np.float32)
    ev = np.asarray(inputs["batch_event"], np.int64)
    feats = np.asarray(inputs["batch_features"], np.float32)
    dists = np.asarray(inputs["batch_distances"], np.float32)
    ctx = np.asarray(inputs["batch_context"], np.int64)

    def mlp(x):
        x = np.maximum(x @ W1.T + b1, 0.0)
        return np.maximum(x @ W2.T + b2, 0.0)

    def l2n(x):
        n = np.linalg.norm(x, axis=-1, keepdims=True)
        return x / np.maximum(n, 1e-12)

    ee = t[ev]                                    # (B, C, E)
    ce = t[ctx]                                   # (B, N, C, E)
    var = np.log1p(np.exp(ee[:, 1, :] @ Wv.T + bv))
    de = np.exp(-(dists * dists) / var)
    extracted = np.concatenate([de, feats], axis=1)
    er = mlp(ee.reshape(B, 1, C * E))
    cr = mlp(ce.reshape(B, N, C * E))
    trans = np.einsum("bmd,bnd->bmn", l2n(er), l2n(cr))
    mus = np.array(MUS, np.float32)
    sig = np.array(SIGMAS, np.float32)
    kk = np.exp(-((trans[..., None] - mus) ** 2) / (2.0 * sig**2))
    kp = np.log(np.clip(kk.sum(axis=2), 1e-10, None)) * 0.01
    allf = np.concatenate([extracted[:, None, :], kp], axis=-1)
    scores = (allf @ Wc.T + bc)[..., 0]
    return (1.0 / (1.0 + np.exp(-scores))).astype(np.float32)


def kernel(**inputs) -> np.ndarray:
    ctx = np.asarray(inputs["batch_context"], np.int64)
    ev = np.asarray(inputs["batch_event"], np.int64)
    fast = True
    for core in range(NCORES):
        sl = slice(core * BC, (core + 1) * BC)
        nuniq = len(np.unique(np.concatenate(
            [ctx[sl].reshape(-1), ev[sl].reshape(-1)])))
        if nuniq > CT:
            fast = False
            break
    if not fast:  # pragma: no cover - impossible for random inputs
        return _numpy_fallback(inputs)

    f8 = ml_dtypes.float8_e4m3fn
    tableq = (np.asarray(inputs["event_table"], np.float32) * XS).astype(f8)
    consts = _prep_fast_consts(inputs)
    nc = _build_fast()
    in_maps = [
        _prep_fast_core(inputs, consts, tableq, core) for core in range(NCORES)
    ]
    res = run_bass_kernel_spmd(nc, in_maps, core_ids=list(range(NCORES)))
    return np.concatenate([r["out"] for r in res.results], axis=0)


if __name__ == "__main__":
    nc = _build_fast()
    print("program built ok")
